# revision 1
# baseline (speedup 1.0000x reference)
"""Trainium2 Bass kernel for nn_Attention_4810363372413.

GQA attention: B=2, S=2048, E=2048, HQ=32, HK=8, D=64, RoPE, no mask
(mask input is all zeros), no 1/sqrt(d) scaling.

Sharding: 8 cores, core c owns kv-head c and q-heads 4c..4c+3
(tensor parallel over heads). Each core computes a partial output
projection over its 4 heads; the host sums the 8 partials.
"""

import os
import sys

sys.path.insert(0, "/opt/trn_rl_repo")

import numpy as np

# Problem constants (hardcoded per contract)
B, S, E = 2, 2048, 2048
HQ, HK, D = 32, 8, 64
NCORES = 8
HL = HQ // NCORES       # 4 local q heads per core
T = B * S               # 4096 tokens total
P = 128

_CACHED = {}


def _build_nc(reps=1):
    import concourse.mybir as mybir
    import concourse.tile as tile
    from concourse import bacc
    from concourse.bass import ts
    from concourse.masks import make_identity

    f32 = mybir.dt.float32
    f32r = mybir.dt.float32r
    bf16 = mybir.dt.bfloat16
    Exp = mybir.ActivationFunctionType.Exp

    nc = bacc.Bacc("TRN2", target_bir_lowering=False, debug=False)

    xt = nc.dram_tensor("xt", [E, T], f32r, kind="ExternalInput").ap()
    wq = nc.dram_tensor("wq", [E, HL * D], f32r, kind="ExternalInput").ap()
    wkv = nc.dram_tensor("wkv", [E, 2 * D], f32r, kind="ExternalInput").ap()
    wo = nc.dram_tensor("wo", [HL * D, E], f32r, kind="ExternalInput").ap()
    cosr = nc.dram_tensor("cosr", [P, S], f32, kind="ExternalInput").ap()
    sinr = nc.dram_tensor("sinr", [P, S], f32, kind="ExternalInput").ap()
    out = nc.dram_tensor("out", [T, E], f32, kind="ExternalOutput").ap()

    EO = E // P   # 16 e-chunks
    TT = 512      # token tile for projections
    NQB = S // 512  # qt blocks of 512 per batch
    NKT = S // P    # 16 key chunks

    def r(ap):
        return ap.bitcast(f32r)

    pools = {}

    def qkv_proj(b, qcomb, kv_t):
        import concourse.bass as _bass

        xtp, mmps = pools["xtp"], pools["mmps"]
        wq_sb, wkv_sb = pools["wq_sb"], pools["wkv_sb"]
        TQ = 256
        xtr = xt.rearrange("(eo p) t -> p eo t", p=P)
        for tt in range(S // TQ):
            # bankA holds both q head-pair chunks (two accumulation groups
            # sharing one PSUM bank: q1's first matmul must come after q0's
            # start=True, which clears the whole bank)
            bankA = mmps.tile([P, 2, TQ], f32, name=f"bkA_{b}_{tt}", tag="mm")
            bankB = mmps.tile([P, TQ], f32, name=f"bkB_{b}_{tt}", tag="mm")
            m0_first = None
            for g in range(EO // 2):
                xc = xtp.tile([P, 2, TQ], f32r, name=f"xc_{b}_{tt}_{g}", tag="xt")
                nc.sync.dma_start(
                    xc[:],
                    xtr[:, 2 * g : 2 * g + 2,
                        b * S + tt * TQ : b * S + (tt + 1) * TQ],
                )
                for j in range(2):
                    e = 2 * g + j
                    st, sp_ = (e == 0), (e == EO - 1)
                    m0 = nc.tensor.matmul(
                        bankA[:, 0], wq_sb[:, e, 0:P], xc[:, j],
                        start=st, stop=sp_,
                    )
                    m1 = nc.tensor.matmul(
                        bankA[:, 1], wq_sb[:, e, P : 2 * P], xc[:, j],
                        start=False, stop=sp_, skip_group_check=True,
                    )
                    if e == 0:
                        _bass._add_dep_helper(
                            m1.ins, m0.ins, sync=True,
                            reason="shared-bank q1 after q0 bank clear",
                        )
                    nc.tensor.matmul(
                        bankB[:], wkv_sb[:, e, :], xc[:, j], start=st, stop=sp_
                    )
            nc.vector.tensor_copy(
                out=qcomb[:, :, ts(tt, TQ)], in_=bankA[:]
            )
            nc.vector.tensor_copy(out=kv_t[:, ts(tt, TQ)], in_=bankB[:])

    def rope(b, qcomb, kv_t):
        qswp, ktmpp = pools["qswp"], pools["ktmpp"]
        cos_sb, sin_sb = pools["cos_sb"], pools["sin_sb"]
        for qi in range(2):
            q_t = qcomb[:, qi]
            for sl in range(S // TT):
                qsw = qswp.tile([P, TT], f32r, name=f"qsw_{b}_{qi}_{sl}", tag="qsw")
                for blk in range(4):
                    srcb = blk ^ 1
                    nc.scalar.dma_start(
                        qsw[blk * 32 : (blk + 1) * 32, :],
                        q_t[srcb * 32 : (srcb + 1) * 32, ts(sl, TT)],
                    )
                nc.vector.tensor_mul(
                    q_t[:, ts(sl, TT)], q_t[:, ts(sl, TT)], cos_sb[:, ts(sl, TT)]
                )
                nc.vector.tensor_mul(qsw[:], qsw[:], sin_sb[:, ts(sl, TT)])
                nc.vector.tensor_add(q_t[:, ts(sl, TT)], q_t[:, ts(sl, TT)], qsw[:])
        for sl in range(S // TT):
            ksw = ktmpp.tile([64, TT], f32r, name=f"ksw_{b}_{sl}", tag="ksw")
            nc.scalar.dma_start(ksw[0:32, :], kv_t[32:64, ts(sl, TT)])
            nc.scalar.dma_start(ksw[32:64, :], kv_t[0:32, ts(sl, TT)])
            nc.vector.tensor_mul(
                kv_t[0:64, ts(sl, TT)],
                kv_t[0:64, ts(sl, TT)],
                cos_sb[0:64, ts(sl, TT)],
            )
            nc.vector.tensor_mul(ksw[:], ksw[:], sin_sb[0:64, ts(sl, TT)])
            nc.vector.tensor_add(
                kv_t[0:64, ts(sl, TT)], kv_t[0:64, ts(sl, TT)], ksw[:]
            )

    def make_v_tiles(b, kv_t):
        mmps, vp, ident = pools["mmps"], pools["vp"], pools["ident"]
        v_tiles = []
        for tch in range(S // P):
            psv = mmps.tile([P, 64], f32r, name=f"psv_{b}_{tch}", tag="mm")
            nc.tensor.matmul(
                psv[:],
                kv_t[64:128, ts(tch, P)],
                ident_r[64:128, 64:128],
                is_transpose=True,
            )
            v_t = vp.tile([P, 65], bf16, name=f"v_{b}_{tch}", tag="v")
            nc.vector.tensor_copy(out=v_t[:, 0:64], in_=psv[:])
            nc.vector.memset(v_t[:, 64:65], 1.0)
            v_tiles.append(v_t)
        return v_tiles

    def scores_exp(b, pair, qtb, q_t, kd_t):
        scps, expsp = pools["scps"], pools["expsp"]
        exps_tiles = []
        for kt in range(NKT):
            sp = scps.tile([P, 1024], f32, name=f"sp_{b}_{pair}_{qtb}_{kt}", tag="sp")
            nc.tensor.matmul(
                sp[:, 0:512],
                kd_t[0:64, ts(kt, P)],
                q_t[0:64, ts(qtb, 512)],
                start=True,
                stop=True,
            )
            nc.tensor.matmul(
                sp[:, 512:1024],
                kd_t[64:128, ts(kt, P)],
                q_t[64:128, ts(qtb, 512)],
                start=True,
                stop=True,
            )
            ex = expsp.tile([P, 1024], bf16, name=f"ex_{b}_{pair}_{qtb}_{kt}", tag="ex")
            nc.scalar.activation(ex[:], sp[:], Exp)
            exps_tiles.append(ex)
        return exps_tiles

    def o_block(b, pair, qtb, hh, exps_tiles, v_tiles, oT_t):
        ops = pools["ops"]
        rzp, onormp, ident_r = pools["rzp"], pools["onormp"], pools["ident_r"]
        onrp = pools["onrp"]
        for qs in range(4):
            po = ops.tile(
                [P, 65], f32, name=f"po_{b}_{pair}_{qtb}_{hh}_{qs}", tag="po"
            )
            for kt in range(NKT):
                nc.tensor.matmul(
                    po[:],
                    exps_tiles[kt][:, hh * 512 + qs * P : hh * 512 + (qs + 1) * P],
                    v_tiles[kt][:],
                    start=(kt == 0),
                    stop=(kt == NKT - 1),
                )
            onr = onrp.tile(
                [P, 65], f32, name=f"onr_{b}_{pair}_{qtb}_{hh}_{qs}", tag="onr"
            )
            nc.vector.tensor_copy(out=onr[:], in_=po[:])
            rz = rzp.tile([P, 1], f32, name=f"rz_{b}_{pair}_{qtb}_{hh}_{qs}", tag="rz")
            nc.vector.reciprocal(rz[:], onr[:, 64:65])
            on = onormp.tile(
                [P, 64], f32r, name=f"on_{b}_{pair}_{qtb}_{hh}_{qs}", tag="on"
            )
            nc.vector.tensor_scalar_mul(on[:], onr[:, 0:64], rz[:])
            pq = ops.tile(
                [64, P], f32r, name=f"pq_{b}_{pair}_{qtb}_{hh}_{qs}", tag="po"
            )
            nc.tensor.matmul(pq[:], on[:], ident_r[:], is_transpose=True)
            nc.vector.tensor_copy(
                out=oT_t[hh * 64 : (hh + 1) * 64,
                         qtb * 512 + qs * P : qtb * 512 + (qs + 1) * P],
                in_=pq[:],
            )

    def out_proj(b, oT_tiles):
        mmps, outp, wo_sb = pools["mmps"], pools["outp"], pools["wo_sb"]
        for tch in range(S // P):
            for eh in range(2):
                os_t = outp.tile([P, E // 2], f32, name=f"os_{b}_{tch}_{eh}", tag="os")
                for ei in range(2):
                    et = eh * 2 + ei
                    ps = mmps.tile(
                        [P, 512], f32, name=f"pso_{b}_{tch}_{et}", tag="mm"
                    )
                    for j in range(2):
                        nc.tensor.matmul(
                            ps[:],
                            oT_tiles[j][:, ts(tch, P)],
                            wo_sb[:, j, ts(et, 512)],
                            start=(j == 0),
                            stop=(j == 1),
                        )
                    nc.vector.tensor_copy(out=os_t[:, ts(ei, 512)], in_=ps[:])
                nc.scalar.dma_start(
                    out[b * S + tch * P : b * S + (tch + 1) * P, ts(eh, E // 2)],
                    os_t[:],
                )

    from contextlib import ExitStack

    with tile.TileContext(nc) as tc:
        with ExitStack() as stk:
            ep = stk.enter_context
            const = ep(tc.tile_pool(name="const", bufs=1))
            xtp = ep(tc.tile_pool(name="xtp", bufs=3))
            qp = ep(tc.tile_pool(name="qp", bufs=2))
            qswp = ep(tc.tile_pool(name="qsw", bufs=2))
            kvp = ep(tc.tile_pool(name="kvp", bufs=1))
            ktmpp = ep(tc.tile_pool(name="ktmp", bufs=2))
            kdp = ep(tc.tile_pool(name="kdp", bufs=2))
            vp = ep(tc.tile_pool(name="vp", bufs=20))
            expsp = ep(tc.tile_pool(name="exps", bufs=24))
            onormp = ep(tc.tile_pool(name="onorm", bufs=6))
            onrp = ep(tc.tile_pool(name="onr", bufs=4))
            rzp = ep(tc.tile_pool(name="rzp", bufs=4))
            otp = ep(tc.tile_pool(name="otp", bufs=2))
            outp = ep(tc.tile_pool(name="outp", bufs=2))
            scps = ep(tc.tile_pool(name="scps", bufs=2, space="PSUM"))
            ops = ep(tc.tile_pool(name="ops", bufs=2, space="PSUM"))
            mmps = ep(tc.tile_pool(name="mmps", bufs=2, space="PSUM"))
            pools.update(
                xtp=xtp, qswp=qswp, ktmpp=ktmpp, vp=vp, expsp=expsp,
                onormp=onormp, onrp=onrp, rzp=rzp, outp=outp, scps=scps, ops=ops, mmps=mmps,
            )
            # ---- constants ----
            ident = const.tile([P, P], f32)
            make_identity(nc, ident)
            ident_r = const.tile([P, P], f32r)
            nc.vector.tensor_copy(out=ident_r[:], in_=ident[:])
            wq_sb = const.tile([P, EO, HL * D], f32r)
            nc.sync.dma_start(wq_sb[:], wq.rearrange("(eo p) m -> p eo m", p=P))
            wkv_sb = const.tile([P, EO, 2 * D], f32r)
            nc.sync.dma_start(wkv_sb[:], wkv.rearrange("(eo p) m -> p eo m", p=P))
            wo_sb = const.tile([P, 2, E], f32r)
            nc.sync.dma_start(wo_sb[:], wo.rearrange("(c p) e -> p c e", p=P))
            cos_sb = const.tile([P, S], f32)
            nc.sync.dma_start(cos_sb[:], cosr)
            sin_sb = const.tile([P, S], f32)
            nc.sync.dma_start(sin_sb[:], sinr)
            pools.update(
                ident=ident, ident_r=ident_r, wq_sb=wq_sb, wkv_sb=wkv_sb, wo_sb=wo_sb,
                cos_sb=cos_sb, sin_sb=sin_sb,
            )

            for bb in range(reps * B):
                b = bb % B
                qcomb = qp.tile([P, 2, S], f32r, name=f"q_{b}", tag="q")
                kv_t = kvp.tile([P, S], f32r, name=f"kv_{b}", tag="kv")
                qkv_proj(b, qcomb, kv_t)
                rope(b, qcomb, kv_t)
                kd_t = kdp.tile([P, S], f32r, name=f"kd_{b}", tag="kd")
                for sl in range(S // TT):
                    nc.scalar.dma_start(
                        kd_t[0:64, ts(sl, TT)], kv_t[0:64, ts(sl, TT)]
                    )
                    nc.scalar.dma_start(
                        kd_t[64:128, ts(sl, TT)], kv_t[0:64, ts(sl, TT)]
                    )
                v_tiles = make_v_tiles(b, kv_t)

                oT_tiles = []
                for pair in range(2):
                    oT_t = otp.tile([P, S], f32r, name=f"oT_{b}_{pair}", tag="oT")
                    oT_tiles.append(oT_t)
                    for qtb in range(NQB):
                        exps_tiles = scores_exp(b, pair, qtb, qcomb[:, pair], kd_t)
                        for hh in range(2):
                            o_block(b, pair, qtb, hh, exps_tiles, v_tiles, oT_t)

                out_proj(b, oT_tiles)

    nc.compile()
    return nc


def _prep_in_maps(inputs):
    x = np.ascontiguousarray(np.asarray(inputs["x"], dtype=np.float32))
    cos = np.asarray(inputs["rope_cos"], dtype=np.float32)
    sin = np.asarray(inputs["rope_sin"], dtype=np.float32)
    Wq = np.asarray(inputs["Wq"], dtype=np.float32)
    Wk = np.asarray(inputs["Wk"], dtype=np.float32)
    Wv = np.asarray(inputs["Wv"], dtype=np.float32)
    Wo = np.asarray(inputs["Wo"], dtype=np.float32)

    xT = np.ascontiguousarray(x.reshape(T, E).T)  # [E, T]
    cosT = np.ascontiguousarray(cos[0, :, 0, :].T)  # [32, S]
    sinT = np.ascontiguousarray(sin[0, :, 0, :].T)  # [32, S]
    cos_rep = np.ascontiguousarray(np.tile(cosT, (4, 1)))  # [128, S]
    sin_rep = np.ascontiguousarray(
        np.tile(np.concatenate([-sinT, sinT], axis=0), (2, 1))
    )  # [128, S] rows: [-s; s; -s; s]

    in_maps = []
    for c in range(NCORES):
        wq_c = np.ascontiguousarray(
            Wq[:, HL * c : HL * (c + 1), :].reshape(E, HL * D)
        )
        wkv_c = np.ascontiguousarray(
            np.concatenate([Wk[:, c, :], Wv[:, c, :]], axis=1)
        )  # [E, 128]
        wo_c = np.ascontiguousarray(
            Wo[HL * c : HL * (c + 1)].reshape(HL * D, E)
        )
        in_maps.append(
            {
                "xt": xT,
                "wq": wq_c,
                "wkv": wkv_c,
                "wo": wo_c,
                "cosr": cos_rep,
                "sinr": sin_rep,
            }
        )
    return in_maps


def kernel(**inputs):
    from concourse.bass_utils import run_bass_kernel_spmd

    if "nc" not in _CACHED:
        _CACHED["nc"] = _build_nc()
    nc = _CACHED["nc"]

    in_maps = _prep_in_maps(inputs)
    trace = bool(int(os.environ.get("ATTN_TRACE", "0")))
    res = run_bass_kernel_spmd(
        nc, in_maps, core_ids=list(range(NCORES)), trace=trace
    )
    _CACHED["last_results"] = res

    acc = res.results[0]["out"].astype(np.float32)
    for c in range(1, NCORES):
        acc = acc + res.results[c]["out"]
    return np.ascontiguousarray(acc.reshape(B, S, E))



# revision 9
# speedup vs baseline: 1.2923x; 1.2923x over previous
"""Trainium2 Bass kernel for nn_Attention_4810363372413.

GQA attention: B=2, S=2048, E=2048, HQ=32, HK=8, D=64, RoPE, no mask
(mask input is all zeros), no 1/sqrt(d) scaling.

Sharding: 8 cores, core c owns kv-head c and q-heads 4c..4c+3
(tensor parallel over heads). Each core computes a partial output
projection over its 4 heads; the host sums the 8 partials.

Schedule (v2): the whole per-core program is software-pipelined around
the Activation engine's exp stream (the secondary bottleneck) and the
PE matmul stream (the primary one):
  - attention runs in (pair, qtb) groups; within a group the kt loop
    interleaves scores(kt) / exp(kt) / AV(kt-1) so ACT never stalls;
  - AV accumulates 8 [128,65] tiles packed into two PSUM banks
    (one bank-clearing start=True per bank + skip_group_check deps);
  - softmax normalization reads PSUM directly (reciprocal +
    tensor_scalar_mul), transposes land in batched [64,512] tiles;
  - out-projection of the previous query block and next-batch QKV
    e-chunks are emitted one unit per kt step as PE "fillers";
  - rope swaps / kd duplication / output writes are DMAs dispatched
    from the otherwise-idle GpSimd (Pool) sequencer.
"""

import os
import sys

sys.path.insert(0, "/opt/trn_rl_repo")

import numpy as np

# Problem constants (hardcoded per contract)
B, S, E = 2, 2048, 2048
HQ, HK, D = 32, 8, 64
NCORES = 8
HL = HQ // NCORES       # 4 local q heads per core
T = B * S               # 4096 tokens total
P = 128

_CACHED = {}


def _build_nc(reps=1):
    import concourse.mybir as mybir
    import concourse.tile as tile
    from concourse import bacc
    from concourse.bass import ts
    from concourse.masks import make_identity
    import concourse.bass as _bass

    f32 = mybir.dt.float32
    f32r = mybir.dt.float32r
    bf16 = mybir.dt.bfloat16
    Exp = mybir.ActivationFunctionType.Exp

    nc = bacc.Bacc("TRN2", target_bir_lowering=False, debug=False)

    xt = nc.dram_tensor("xt", [E, T], f32r, kind="ExternalInput").ap()
    wq = nc.dram_tensor("wq", [E, HL * D], f32r, kind="ExternalInput").ap()
    wkv = nc.dram_tensor("wkv", [E, 2 * D], f32r, kind="ExternalInput").ap()
    wo = nc.dram_tensor("wo", [HL * D, E], f32r, kind="ExternalInput").ap()
    cosr = nc.dram_tensor("cosr", [P, S], f32, kind="ExternalInput").ap()
    sinr = nc.dram_tensor("sinr", [P, S], f32, kind="ExternalInput").ap()
    out = nc.dram_tensor("out", [T, E], f32, kind="ExternalOutput").ap()

    EO = E // P     # 16 e-chunks
    TQ = 256        # token tile for projections
    NTT = S // TQ   # 8 projection tiles per batch
    NQB = S // 512  # 4 query blocks of 512 per batch
    NKT = S // P    # 16 key chunks
    TT = 512        # rope slice

    pools = {}
    st = {}  # per-rep mutable state: qcomb/kv/kd/v/oT handles

    xtr = xt.rearrange("(eo p) t -> p eo t", p=P)

    # ---------------- emission helpers ----------------

    def qkv_xc_unit(b, tt, g, tag):
        """One x chunk (2 e-slices) + its 6 projection matmuls."""
        xtp, mmps = pools["xtp"], pools["mmps"]
        wq_sb, wkv_sb = pools["wq_sb"], pools["wkv_sb"]
        if g == 0:
            st[("bankA", b, tt)] = mmps.tile(
                [P, 2, TQ], f32, name=f"bkA_{tag}", tag="mm"
            )
            st[("bankB", b, tt)] = mmps.tile(
                [P, TQ], f32, name=f"bkB_{tag}", tag="mm"
            )
            st[("m0first", b, tt)] = None
        bankA = st[("bankA", b, tt)]
        bankB = st[("bankB", b, tt)]
        xc = xtp.tile([P, 2, TQ], f32r, name=f"xc_{tag}", tag="xt")
        nc.sync.dma_start(
            xc[:],
            xtr[:, 2 * g : 2 * g + 2, b * S + tt * TQ : b * S + (tt + 1) * TQ],
        )
        for j in range(2):
            e = 2 * g + j
            stt, spp = (e == 0), (e == EO - 1)
            m0 = nc.tensor.matmul(
                bankA[:, 0], wq_sb[:, e, 0:P], xc[:, j], start=stt, stop=spp
            )
            m1 = nc.tensor.matmul(
                bankA[:, 1], wq_sb[:, e, P : 2 * P], xc[:, j],
                start=False, stop=spp, skip_group_check=True,
            )
            if e == 0:
                st[("m0first", b, tt)] = m0
                _bass._add_dep_helper(
                    m1.ins, m0.ins, sync=True,
                    reason="shared-bank q1 after q0 bank clear",
                )
            nc.tensor.matmul(
                bankB[:], wkv_sb[:, e, :], xc[:, j], start=stt, stop=spp
            )
        if g == EO // 2 - 1:
            qcomb, kv_t = st[("q", b)], st[("kv", b)]
            nc.vector.tensor_copy(out=qcomb[:, :, ts(tt, TQ)], in_=bankA[:])
            nc.vector.tensor_copy(out=kv_t[:, ts(tt, TQ)], in_=bankB[:])

    def alloc_qkv(b):
        st[("q", b)] = pools["qp"].tile([P, 2, S], f32r, name=f"q_{b}", tag="q")
        st[("kv", b)] = pools["kvp"].tile([P, S], f32r, name=f"kv_{b}", tag="kv")
        st[("kd", b)] = pools["kdp"].tile([P, S], f32r, name=f"kd_{b}", tag="kd")
        st[("v", b)] = [None] * NKT

    def rope_q(b, sl):
        qswp = pools["qswp"]
        cos_sb, sin_sb = pools["cos_sb"], pools["sin_sb"]
        q2 = st[("q", b)]
        qsw = qswp.tile([P, 2, TT], f32r, name=f"qsw_{b}_{sl}", tag="qsw")
        for blk in range(4):
            srcb = blk ^ 1
            nc.gpsimd.dma_start(
                qsw[blk * 32 : (blk + 1) * 32, :, :],
                q2[srcb * 32 : (srcb + 1) * 32, :, ts(sl, TT)],
            )
        for qi in range(2):
            q_t = q2[:, qi]
            nc.vector.tensor_mul(
                q_t[:, ts(sl, TT)], q_t[:, ts(sl, TT)], cos_sb[:, ts(sl, TT)]
            )
            nc.vector.tensor_mul(
                qsw[:, qi], qsw[:, qi], sin_sb[:, ts(sl, TT)]
            )
            nc.vector.tensor_add(
                q_t[:, ts(sl, TT)], q_t[:, ts(sl, TT)], qsw[:, qi]
            )

    def rope_k_kd(b, sl):
        ktmpp = pools["ktmpp"]
        cos_sb, sin_sb = pools["cos_sb"], pools["sin_sb"]
        kv_t, kd_t = st[("kv", b)], st[("kd", b)]
        ksw = ktmpp.tile([64, TT], f32r, name=f"ksw_{b}_{sl}", tag="ksw")
        nc.gpsimd.dma_start(ksw[0:32, :], kv_t[32:64, ts(sl, TT)])
        nc.gpsimd.dma_start(ksw[32:64, :], kv_t[0:32, ts(sl, TT)])
        nc.vector.tensor_mul(
            kv_t[0:64, ts(sl, TT)], kv_t[0:64, ts(sl, TT)],
            cos_sb[0:64, ts(sl, TT)],
        )
        nc.vector.tensor_mul(ksw[:], ksw[:], sin_sb[0:64, ts(sl, TT)])
        nc.vector.tensor_add(
            kv_t[0:64, ts(sl, TT)], kv_t[0:64, ts(sl, TT)], ksw[:]
        )
        nc.gpsimd.dma_start(kd_t[0:64, ts(sl, TT)], kv_t[0:64, ts(sl, TT)])
        nc.gpsimd.dma_start(kd_t[64:128, ts(sl, TT)], kv_t[0:64, ts(sl, TT)])

    def v_chunk(b, tch):
        mmps, vp = pools["mmps"], pools["vp"]
        ident_r = pools["ident_r"]
        kv_t = st[("kv", b)]
        psv = mmps.tile([P, 64], f32r, name=f"psv_{b}_{tch}", tag="mm")
        nc.tensor.matmul(
            psv[:], kv_t[64:128, ts(tch, P)], ident_r[64:128, 64:128],
            is_transpose=True,
        )
        v_t = vp.tile([P, 65], bf16, name=f"v_{b}_{tch}", tag="v")
        nc.vector.tensor_copy(out=v_t[:, 0:64], in_=psv[:])
        nc.vector.memset(v_t[:, 64:65], 1.0)
        st[("v", b)][tch] = v_t

    def rope_and_v(b, sl):
        rope_q(b, sl)
        rope_k_kd(b, sl)
        for tch in range(4 * sl, 4 * sl + 4):
            v_chunk(b, tch)

    def scores_exp(b, pair, qtb, kt, tag):
        scps, expsp = pools["scps"], pools["expsp"]
        kd_t, qcomb = st[("kd", b)], st[("q", b)]
        sp = scps.tile([P, 1024], f32, name=f"sp_{tag}", tag="sp")
        nc.tensor.matmul(
            sp[:, 0:512], kd_t[0:64, ts(kt, P)],
            qcomb[0:64, pair, ts(qtb, 512)], start=True, stop=True,
        )
        nc.tensor.matmul(
            sp[:, 512:1024], kd_t[64:128, ts(kt, P)],
            qcomb[64:128, pair, ts(qtb, 512)], start=True, stop=True,
        )
        ex = expsp.tile([P, 1024], bf16, name=f"ex_{tag}", tag="ex")
        nc.scalar.activation(ex[:], sp[:], Exp)
        return ex

    def av_step(b, kt, ex, banks, firsts):
        v_t = st[("v", b)][kt]
        for hh in range(2):
            bank = banks[hh]
            for qs in range(4):
                first = kt == 0 and qs == 0
                m = nc.tensor.matmul(
                    bank[:, qs * 65 : qs * 65 + 65],
                    ex[:, hh * 512 + qs * P : hh * 512 + (qs + 1) * P],
                    v_t[:],
                    start=first,
                    stop=(kt == NKT - 1),
                    skip_group_check=not first,
                )
                if first:
                    firsts[hh] = m
                elif kt == 0:
                    _bass._add_dep_helper(
                        m.ins, firsts[hh].ins, sync=True,
                        reason="AV bank-packed accum after bank clear",
                    )

    def finish_group(b, pair, qtb, banks, tag):
        """Normalize the 8 packed AV accumulators, transpose, store to oT."""
        rzp, onp, mmps = pools["rzp"], pools["onp"], pools["mmps"]
        ident_r = pools["ident_r"]
        oT_t = st[("oT", b, pair)]
        for hh in range(2):
            bank = banks[hh]
            rz = rzp.tile([P, 4], f32, name=f"rz_{tag}_{hh}", tag="rz")
            # strided gather of the 4 denominator columns (65th of each acc)
            nc.vector.reciprocal(rz[:], bank[:, 64:260:65])
            on4 = onp.tile([P, 4, 64], f32r, name=f"on_{tag}_{hh}", tag="on")
            for qs in range(4):
                nc.vector.tensor_scalar_mul(
                    on4[:, qs], bank[:, qs * 65 : qs * 65 + 64],
                    rz[:, qs : qs + 1],
                )
            pq4 = mmps.tile([64, 512], f32r, name=f"pq_{tag}_{hh}", tag="mm")
            for qs in range(4):
                nc.tensor.matmul(
                    pq4[:, ts(qs, P)], on4[:, qs], ident_r[:],
                    is_transpose=True,
                )
            nc.vector.tensor_copy(
                out=oT_t[hh * 64 : (hh + 1) * 64, ts(qtb, 512)], in_=pq4[:]
            )

    def outproj_unit(b, qtb, s, tag):
        """One (tch, et) slice of the output projection for query block qtb."""
        mmps, outp, wo_sb = pools["mmps"], pools["outp"], pools["wo_sb"]
        tch = qtb * 4 + s // 4
        et = s % 4
        if et % 2 == 0:
            st[("os", b, tch)] = outp.tile(
                [P, 1024], f32, name=f"os_{tag}_{tch}_{et}", tag="os"
            )
        os_t = st[("os", b, tch)]
        ps = mmps.tile([P, 512], f32, name=f"pso_{tag}_{tch}_{et}", tag="mm")
        for j in range(2):
            nc.tensor.matmul(
                ps[:], st[("oT", b, j)][:, ts(tch, P)],
                wo_sb[:, j, ts(et, 512)], start=(j == 0), stop=(j == 1),
            )
        nc.vector.tensor_copy(out=os_t[:, ts(et % 2, 512)], in_=ps[:])
        if et % 2 == 1:
            nc.sync.dma_start(
                out[
                    b * S + tch * P : b * S + (tch + 1) * P,
                    (et // 2) * 1024 : (et // 2 + 1) * 1024,
                ],
                os_t[:],
            )

    def attention_group(b, pair, qtb, fillers, tag):
        ops = pools["ops"]
        if ("oT", b, pair) not in st or st[("oT", b, pair)] is None:
            st[("oT", b, pair)] = pools["otp"].tile(
                [P, S], f32r, name=f"oT_{b}_{pair}_{tag}", tag="oT"
            )
        banks = [
            ops.tile([P, 512], f32, name=f"avA_{tag}", tag="po"),
            ops.tile([P, 512], f32, name=f"avB_{tag}", tag="po"),
        ]
        firsts = [None, None]
        exs = []
        for step in range(NKT + 1):
            if step < NKT:
                exs.append(scores_exp(b, pair, qtb, step, f"{tag}_{step}"))
            if step > 0:
                av_step(b, step - 1, exs[step - 1], banks, firsts)
                exs[step - 1] = None
            if fillers:
                fillers.pop(0)()
        finish_group(b, pair, qtb, banks, tag)

    # ---------------- program ----------------

    from contextlib import ExitStack

    with tile.TileContext(nc) as tc:
        with ExitStack() as stk:
            ep = stk.enter_context
            const = ep(tc.tile_pool(name="const", bufs=1))
            xtp = ep(tc.tile_pool(name="xtp", bufs=4))
            qp = ep(tc.tile_pool(name="qp", bufs=2))
            qswp = ep(tc.tile_pool(name="qsw", bufs=2))
            kvp = ep(tc.tile_pool(name="kvp", bufs=2))
            ktmpp = ep(tc.tile_pool(name="ktmp", bufs=2))
            kdp = ep(tc.tile_pool(name="kdp", bufs=2))
            vp = ep(tc.tile_pool(name="vp", bufs=36))
            expsp = ep(tc.tile_pool(name="exps", bufs=6))
            onp = ep(tc.tile_pool(name="onp", bufs=4))
            rzp = ep(tc.tile_pool(name="rzp", bufs=4))
            otp = ep(tc.tile_pool(name="otp", bufs=3))
            outp = ep(tc.tile_pool(name="outp", bufs=2))
            scps = ep(tc.tile_pool(name="scps", bufs=2, space="PSUM"))
            ops = ep(tc.tile_pool(name="ops", bufs=2, space="PSUM"))
            mmps = ep(tc.tile_pool(name="mmps", bufs=2, space="PSUM"))
            pools.update(
                xtp=xtp, qp=qp, qswp=qswp, kvp=kvp, ktmpp=ktmpp, kdp=kdp,
                vp=vp, expsp=expsp, onp=onp, rzp=rzp, otp=otp, outp=outp,
                scps=scps, ops=ops, mmps=mmps,
            )
            # ---- constants ----
            ident = const.tile([P, P], f32)
            make_identity(nc, ident)
            ident_r = const.tile([P, P], f32r)
            nc.vector.tensor_copy(out=ident_r[:], in_=ident[:])
            wq_sb = const.tile([P, EO, HL * D], f32r)
            nc.sync.dma_start(wq_sb[:], wq.rearrange("(eo p) m -> p eo m", p=P))
            wkv_sb = const.tile([P, EO, 2 * D], f32r)
            nc.sync.dma_start(wkv_sb[:], wkv.rearrange("(eo p) m -> p eo m", p=P))
            wo_sb = const.tile([P, 2, E], f32r)
            nc.sync.dma_start(wo_sb[:], wo.rearrange("(c p) e -> p c e", p=P))
            cos_sb = const.tile([P, S], f32)
            nc.sync.dma_start(cos_sb[:], cosr)
            sin_sb = const.tile([P, S], f32)
            nc.sync.dma_start(sin_sb[:], sinr)
            pools.update(
                ident=ident, ident_r=ident_r, wq_sb=wq_sb, wkv_sb=wkv_sb,
                wo_sb=wo_sb, cos_sb=cos_sb, sin_sb=sin_sb,
            )

            for rep in range(reps):
                rt = f"r{rep}"
                # startup: full qkv+rope+v for batch 0
                alloc_qkv(0)
                for tt in range(NTT):
                    for g in range(EO // 2):
                        qkv_xc_unit(0, tt, g, f"{rt}_b0_{tt}_{g}")
                for sl in range(S // TT):
                    rope_and_v(0, sl)

                for bb in range(B):
                    st[("oT", bb, 0)] = None
                    st[("oT", bb, 1)] = None

                for b in range(B):
                    for qtb in range(NQB):
                        for pair in range(2):
                            tag = f"{rt}_{b}_{pair}_{qtb}"
                            fillers = []
                            if pair == 0:
                                # out-projection of the previous query block
                                pb, pq_ = (b, qtb - 1) if qtb > 0 else (
                                    (b - 1, NQB - 1) if b > 0 else (None, None)
                                )
                                if pb is not None:
                                    fillers = [
                                        (lambda s, pb=pb, pq_=pq_: lambda:
                                         outproj_unit(pb, pq_, s, rt))(s)
                                        for s in range(16)
                                    ]
                            else:
                                if b == 0:
                                    # next batch's qkv, two tiles per group
                                    if qtb == 0:
                                        alloc_qkv(1)
                                    fillers = [
                                        (lambda tt, g: lambda:
                                         qkv_xc_unit(
                                             1, tt, g, f"{rt}_b1_{tt}_{g}"
                                         ))(2 * qtb + (u // 8), u % 8)
                                        for u in range(16)
                                    ]
                            attention_group(b, pair, qtb, fillers, tag)
                            if b == 0 and pair == 1:
                                rope_and_v(1, qtb)
                # tail: out-projection of the last query block
                for s in range(16):
                    outproj_unit(B - 1, NQB - 1, s, rt)

    nc.compile()
    return nc


def _prep_in_maps(inputs):
    x = np.ascontiguousarray(np.asarray(inputs["x"], dtype=np.float32))
    cos = np.asarray(inputs["rope_cos"], dtype=np.float32)
    sin = np.asarray(inputs["rope_sin"], dtype=np.float32)
    Wq = np.asarray(inputs["Wq"], dtype=np.float32)
    Wk = np.asarray(inputs["Wk"], dtype=np.float32)
    Wv = np.asarray(inputs["Wv"], dtype=np.float32)
    Wo = np.asarray(inputs["Wo"], dtype=np.float32)

    xT = np.ascontiguousarray(x.reshape(T, E).T)  # [E, T]
    cosT = np.ascontiguousarray(cos[0, :, 0, :].T)  # [32, S]
    sinT = np.ascontiguousarray(sin[0, :, 0, :].T)  # [32, S]
    cos_rep = np.ascontiguousarray(np.tile(cosT, (4, 1)))  # [128, S]
    sin_rep = np.ascontiguousarray(
        np.tile(np.concatenate([-sinT, sinT], axis=0), (2, 1))
    )  # [128, S] rows: [-s; s; -s; s]

    in_maps = []
    for c in range(NCORES):
        wq_c = np.ascontiguousarray(
            Wq[:, HL * c : HL * (c + 1), :].reshape(E, HL * D)
        )
        wkv_c = np.ascontiguousarray(
            np.concatenate([Wk[:, c, :], Wv[:, c, :]], axis=1)
        )  # [E, 128]
        wo_c = np.ascontiguousarray(
            Wo[HL * c : HL * (c + 1)].reshape(HL * D, E)
        )
        in_maps.append(
            {
                "xt": xT,
                "wq": wq_c,
                "wkv": wkv_c,
                "wo": wo_c,
                "cosr": cos_rep,
                "sinr": sin_rep,
            }
        )
    return in_maps


def kernel(**inputs):
    from concourse.bass_utils import run_bass_kernel_spmd

    if "nc" not in _CACHED:
        _CACHED["nc"] = _build_nc()
    nc = _CACHED["nc"]

    in_maps = _prep_in_maps(inputs)
    trace = bool(int(os.environ.get("ATTN_TRACE", "0")))
    res = run_bass_kernel_spmd(
        nc, in_maps, core_ids=list(range(NCORES)), trace=trace
    )
    _CACHED["last_results"] = res

    acc = res.results[0]["out"].astype(np.float32)
    for c in range(1, NCORES):
        acc = acc + res.results[c]["out"]
    return np.ascontiguousarray(acc.reshape(B, S, E))
